# revision 14
# baseline (speedup 1.0000x reference)
"""GCN ActorNetwork on 8 TRN2 NeuronCores (Bass/Tile).

Graph-partitioned by destination (12500 nodes/core). Per core and layer:
table g=(x@W)*dinv -> DRAM -> AllGather; per-edge rows fetched with 4-queue
dma_gather (int16 windowed idx); segment sums via PE matmul against
on-device-built one-hot matrices accumulated in PSUM per 128-dest group
(race-free, no scatter). Mean-pool partial + AllReduce; replicated LSTM head.
Edge bookkeeping (CSR grouping/tiling/padding/windowing) is host numpy.
"""
import os
import numpy as np
import concourse.bass as bass
import concourse.bacc as bacc
import concourse.tile as tile
import concourse.mybir as mybir
from concourse.bass_utils import run_bass_kernel_spmd

F32 = mybir.dt.float32
I16 = mybir.dt.int16
AF = mybir.ActivationFunctionType
OP = mybir.AluOpType

NCORES = 8
N_NODES = 100000
NPC = 12500
NPAD = 12544          # 98*128 table rows per core
NT = 98
DG = 128              # dests per group
NGROUPS = 98          # ceil(12500/128)
NFREE = NGROUPS * DG  # 12544
WIN = 4
WROWS = 2 * NPAD      # 25088 (< 32768, int16-safe)
GPB = 8               # groups per block
NBLK = (NGROUPS + GPB - 1) // GPB  # 7 blocks (last has 2 groups)
ADIM, LSTM_H = 64, 256


def _host_prep(x_graph, edge_index):
    loop = np.arange(N_NODES, dtype=np.int64)
    row = np.concatenate([edge_index[0].astype(np.int64), loop])
    col = np.concatenate([edge_index[1].astype(np.int64), loop])
    deg = np.bincount(col, minlength=N_NODES).astype(np.float32)
    # per core, per (group, window): source gather rows + dest cols
    per_core = []
    for k in range(NCORES):
        m = (col >= k * NPC) & (col < (k + 1) * NPC)
        r, c = row[m], col[m] - k * NPC
        ln = r % NPC
        grow = (r // NPC) * NPAD + (ln % 128) * NT + (ln // 128)
        w = (r // NPC) // 2
        g = c // DG
        key = g * WIN + w
        order = np.argsort(key, kind="stable")
        r, c, w, g, grow = r[order], c[order], w[order], g[order], grow[order]
        wrel = grow - w * WROWS
        bounds = np.searchsorted(key[order], np.arange(NGROUPS * WIN + 1))
        gw = {}
        for gi in range(NGROUPS):
            for wi in range(WIN):
                a, b = bounds[gi * WIN + wi], bounds[gi * WIN + wi + 1]
                gw[(gi, wi)] = (wrel[a:b], (c[a:b] - gi * DG))
        per_core.append(gw)
    # unified tile counts
    tiles_gw = np.zeros((NGROUPS, WIN), np.int64)
    for gi in range(NGROUPS):
        for wi in range(WIN):
            mx = max(per_core[k][(gi, wi)][0].size for k in range(NCORES))
            tiles_gw[gi, wi] = max((mx + 127) // 128, 1)
    # layout: block b -> window w -> group g(in block) -> tiles
    slot_of = {}   # (g, w) -> starting slot (global)
    blk_meta = []  # per block: (slot0, slots, [(call_off16, call_n)]*WIN)
    soff = 0
    off16 = 0
    for b in range(NBLK):
        gs = range(b * GPB, min((b + 1) * GPB, NGROUPS))
        s0 = soff
        calls = []
        for wi in range(WIN):
            n = int(sum(tiles_gw[gi, wi] for gi in gs)) * 128
            calls.append((off16, n))
            for gi in gs:
                slot_of[(gi, wi)] = soff
                soff += int(tiles_gw[gi, wi])
            off16 += n // 16
        blk_meta.append((s0, soff - s0, calls))
    NSLOTS = soff
    W16 = off16
    # per-core gidx / didx
    gidx_l, didx_l = [], []
    for k in range(NCORES):
        gvals = np.zeros(NSLOTS * 128, np.int16)
        dvals = np.full(NSLOTS * 128, -1.0, np.float32)
        for gi in range(NGROUPS):
            for wi in range(WIN):
                ws, cs = per_core[k][(gi, wi)]
                s0 = slot_of[(gi, wi)] * 128
                gvals[s0:s0 + ws.size] = ws.astype(np.int16)
                dvals[s0:s0 + cs.size] = cs.astype(np.float32)
        gt = np.zeros((128, W16), np.int16)
        # wrap per call region
        for b in range(NBLK):
            s0, slots, calls = blk_meta[b]
            for wi in range(WIN):
                o16, n = calls[wi]
                if n == 0:
                    continue
                base = (s0 + sum(
                    int(tiles_gw[gi, w2])
                    for w2 in range(wi)
                    for gi in range(b * GPB, min((b + 1) * GPB, NGROUPS))
                )) * 128
                arr = gvals[base:base + n]
                blkw = arr.reshape(n // 16, 16).T
                gt[32 * wi:32 * wi + 16, o16:o16 + n // 16] = blkw
                gt[32 * wi + 16:32 * wi + 32, o16:o16 + n // 16] = blkw
        dt = dvals.reshape(NSLOTS, 128).T.copy()  # didx[p, slot]
        gidx_l.append(gt)
        didx_l.append(dt)
    degr = []
    degf = []
    nvec = np.arange(NPC)
    for k in range(NCORES):
        dg = deg[k * NPC:(k + 1) * NPC]
        dr = np.ones((128, NT), np.float32)
        dr[nvec % 128, nvec // 128] = dg
        df = np.ones((1, NFREE), np.float32)
        df[0, :NPC] = dg
        degr.append(dr)
        degf.append(np.tile(df, (32, 1)))
    return dict(gidx=gidx_l, didx=didx_l, degr=degr, degf=degf,
                blk_meta=blk_meta, tiles_gw=tiles_gw, slot_of=slot_of,
                NSLOTS=NSLOTS, W16=W16)


def kernel(**inputs):
    xg = np.asarray(inputs["x_graph"], np.float32)
    ei = np.asarray(inputs["edge_index"])
    pp = _host_prep(xg, ei)
    b_ih = np.asarray(inputs["b_ih"], np.float32)
    b_hh = np.asarray(inputs["b_hh"], np.float32)
    in_maps = []
    for k in range(NCORES):
        xs = xg[k * NPC:(k + 1) * NPC]
        xt = np.zeros((64, NPAD), np.float32)
        xt[:, :NPC] = xs.T
        in_maps.append({
            "xt": xt, "gidx": pp["gidx"][k], "didx": pp["didx"][k],
            "degr": pp["degr"][k], "degf": pp["degf"][k],
            "w1": np.asarray(inputs["W1"], np.float32),
            "w2": np.asarray(inputs["W2"], np.float32),
            "b1": np.asarray(inputs["b1"], np.float32).reshape(32, 1),
            "b2": np.asarray(inputs["b2"], np.float32).reshape(32, 1),
            "iota": np.tile(np.arange(DG, dtype=np.float32), (128, 1)),
            "wihT": np.asarray(inputs["W_ih"], np.float32).T.copy(),
            "whh0": np.asarray(inputs["W_hh"], np.float32).T[0:128].copy(),
            "whh1": np.asarray(inputs["W_hh"], np.float32).T[128:256].copy(),
            "bhx": (b_ih + b_hh).reshape(8, 128).T.copy(),
            "wfc": np.asarray(inputs["W_fc"], np.float32),
            "bfc": np.asarray(inputs["b_fc"], np.float32).reshape(1, 64),
            "xstate_c": np.asarray(inputs["x_state"],
                                   np.float32).reshape(64, 1),
            "h0c": np.asarray(inputs["h0"], np.float32).reshape(256, 1),
            "c0t": np.asarray(inputs["c0"], np.float32).reshape(256)
                     .reshape(2, 128).T.copy(),
            "ident": np.eye(64, dtype=np.float32),
        })
    nc = _build_bass(pp)
    res = run_bass_kernel_spmd(nc, in_maps, list(range(NCORES)),
                               trace=bool(os.environ.get("KERNEL_TRACE")))
    if res.exec_time_ns is not None:
        print(f"HW exec time: {res.exec_time_ns} ns")
    r0 = res.results[0]
    return (r0["out_lp"].reshape(1, 1, ADIM).astype(np.float32),
            r0["out_h"].reshape(1, 1, LSTM_H).astype(np.float32),
            r0["out_c"].reshape(1, 1, LSTM_H).astype(np.float32))


def _build_bass(pp):
    NSLOTS, W16 = pp["NSLOTS"], pp["W16"]
    blk_meta, tiles_gw = pp["blk_meta"], pp["tiles_gw"]
    nc = bacc.Bacc("TRN2", target_bir_lowering=False, debug=False,
                   num_devices=NCORES, num_swdge_queues=4)
    d = {}
    for name, shape in [
        ("xt", [64, NPAD]), ("degr", [128, NT]), ("degf", [32, NFREE]),
        ("w1", [64, 32]), ("w2", [32, 32]), ("b1", [32, 1]), ("b2", [32, 1]),
        ("iota", [128, DG]), ("didx", [128, NSLOTS]),
        ("wihT", [96, 1024]), ("whh0", [128, 1024]), ("whh1", [128, 1024]),
        ("bhx", [128, 8]), ("wfc", [256, 64]), ("bfc", [1, 64]),
        ("xstate_c", [64, 1]), ("h0c", [256, 1]), ("c0t", [128, 2]),
        ("ident", [64, 64]),
    ]:
        d[name] = nc.dram_tensor(name, shape, F32, kind="ExternalInput")
    d["gidx"] = nc.dram_tensor("gidx", [128, W16], I16, kind="ExternalInput")
    out_lp = nc.dram_tensor("out_lp", [1, ADIM], F32, kind="ExternalOutput")
    out_h = nc.dram_tensor("out_h", [1, LSTM_H], F32, kind="ExternalOutput")
    out_c = nc.dram_tensor("out_c", [1, LSTM_H], F32, kind="ExternalOutput")
    tbl_loc = [nc.dram_tensor(f"tbl_loc{l}", [NPAD, 64], F32) for l in (0, 1)]
    tbl_glob = [nc.dram_tensor(f"tbl_glob{l}", [NCORES * NPAD, 64], F32,
                               addr_space="Shared") for l in (0, 1)]
    pr_in = nc.dram_tensor("pr_in", [32, 1], F32)
    pr_out = nc.dram_tensor("pr_out", [32, 1], F32, addr_space="Shared")

    with tile.TileContext(nc) as tc:
        with (
            tc.tile_pool(name="big", bufs=1) as big,
            tc.tile_pool(name="msgp", bufs=2) as msgp,
            tc.tile_pool(name="ohp", bufs=1) as ohp,
            tc.tile_pool(name="psg", bufs=2, space="PSUM") as psg,
            tc.tile_pool(name="psn", bufs=1, space="PSUM") as psn,
            tc.tile_pool(name="small", bufs=1) as small,
        ):
            sb = {}
            for name, shape, dt in [
                ("didx", [128, NSLOTS], F32), ("iota", [128, DG], F32),
                ("degr", [128, NT], F32),
                ("w1", [64, 32], F32), ("w2", [32, 32], F32),
                ("b1", [32, 1], F32), ("b2", [32, 1], F32),
            ]:
                sb[name] = big.tile(shape, dt, name=f"{name}_sb", tag=name)
                nc.sync.dma_start(out=sb[name][:], in_=d[name][:, :])
            dinv_r = small.tile([128, NT], F32, name="dinvr")
            nc.scalar.activation(dinv_r[:], sb["degr"][:], AF.Sqrt)
            nc.vector.reciprocal(dinv_r[:], dinv_r[:])
            xwork = big.tile([32, NFREE], F32, name="xwork_sb")

            for layer in (0, 1):
                ws = sb["w1"] if layer == 0 else sb["w2"]
                bs = sb["b1"] if layer == 0 else sb["b2"]
                kdim = 64 if layer == 0 else 32
                tblv = tbl_loc[layer].ap().rearrange("(p s) e -> p s e", p=128)
                for t in range(NT):
                    if layer == 0:
                        xtile = msgp.tile([64, 128], F32, name=f"xt{t}",
                                          tag="xtile")
                        nc.sync.dma_start(out=xtile[:],
                                          in_=d["xt"][:, t * 128:(t + 1) * 128])
                        ls = xtile[:]
                    else:
                        ls = xwork[0:kdim, t * 128:(t + 1) * 128]
                    pr = psn.tile([128, 32], F32, name=f"pr{layer}_{t}",
                                  tag="prow")
                    nc.tensor.matmul(out=pr[:], lhsT=ls, rhs=ws[0:kdim, :],
                                     start=True, stop=True)
                    srow = msgp.tile([128, 32], F32, name=f"sr{layer}_{t}",
                                     tag="srow")
                    nc.vector.tensor_scalar(
                        out=srow[:], in0=pr[:],
                        scalar1=dinv_r[:, t:t + 1], scalar2=None, op0=OP.mult)
                    nc.sync.dma_start(out=tblv[:, t, 0:32], in_=srow[:])
                nc.gpsimd.collective_compute(
                    "AllGather", OP.bypass,
                    replica_groups=[list(range(NCORES))],
                    ins=[tbl_loc[layer][:, :]], outs=[tbl_glob[layer][:, :]])
                for b in range(NBLK):
                    s0, slots, calls = blk_meta[b]
                    gs = list(range(b * GPB, min((b + 1) * GPB, NGROUPS)))
                    msg = msgp.tile([128, slots, 64], F32,
                                    name=f"m{layer}_{b}", tag="msg")
                    ncols16 = sum(n for _, n in calls) // 16
                    gix = ohp.tile([128, ncols16], I16,
                                   name=f"gx{layer}_{b}", tag="gix")
                    nc.sync.dma_start(
                        out=gix[:],
                        in_=d["gidx"][:, calls[0][0]:calls[0][0] + ncols16])
                    so = 0
                    go = 0
                    for wi in range(WIN):
                        o16, n = calls[wi]
                        if n == 0:
                            continue
                        nc.gpsimd.dma_gather(
                            out_ap=msg[:, so:so + n // 128, :],
                            in_ap=tbl_glob[layer][wi * WROWS:(wi + 1) * WROWS,
                                                  :],
                            idxs_ap=gix[:, go:go + n // 16],
                            num_idxs=n, num_idxs_reg=n, elem_size=64,
                            single_packet=False, queue_num=wi)
                        so += n // 128
                        go += n // 16
                    pst = [psg.tile([32, 512], F32, name=f"pb{layer}_{b}_{i}",
                                    tag=f"pseg{i % 2}") for i in range(2)]
                    cnt = {gi: 0 for gi in gs}
                    tot = {gi: int(tiles_gw[gi].sum()) for gi in gs}
                    # slot order within block: (w, g, tile)
                    order = []
                    for wi in range(WIN):
                        for gi in gs:
                            for _ in range(int(tiles_gw[gi, wi])):
                                order.append(gi)
                    for sub in range(0, slots, 32):
                        nsub = min(32, slots - sub)
                        oh = ohp.tile([128, 32, DG], F32,
                                      name=f"oh{layer}_{b}_{sub}", tag="oh")
                        nc.vector.tensor_tensor(
                            out=oh[:, 0:nsub, :],
                            in0=sb["didx"][:, s0 + sub:s0 + sub + nsub]
                                .to_broadcast([128, nsub, DG]),
                            in1=sb["iota"][:, None, :].to_broadcast(
                                [128, nsub, DG]),
                            op=OP.is_equal)
                        for j in range(nsub):
                            gi = order[sub + j]
                            gl = gi - b * GPB
                            ps = pst[gl // 4]
                            off = (gl % 4) * 128
                            nc.tensor.matmul(
                                out=ps[:, off:off + 128],
                                lhsT=msg[:, sub + j, 0:32],
                                rhs=oh[:, j, :],
                                start=(cnt[gi] == 0),
                                stop=(cnt[gi] == tot[gi] - 1))
                            cnt[gi] += 1
                    for gl4 in range((len(gs) + 3) // 4):
                        ncols = min(len(gs) - gl4 * 4, 4) * 128
                        c0 = (b * GPB + gl4 * 4) * DG
                        nc.vector.tensor_copy(
                            out=xwork[:, c0:c0 + ncols],
                            in_=pst[gl4][:, 0:ncols])
                for qc in range(8):
                    c0 = qc * (NFREE // 8)
                    dfc = msgp.tile([32, NFREE // 8], F32,
                                    name=f"df{layer}_{qc}", tag="dfc")
                    nc.sync.dma_start(out=dfc[:],
                                      in_=d["degf"][:, c0:c0 + NFREE // 8])
                    nc.scalar.activation(dfc[:], dfc[:], AF.Sqrt)
                    nc.vector.reciprocal(dfc[:], dfc[:])
                    nc.vector.tensor_tensor(
                        out=xwork[:, c0:c0 + NFREE // 8],
                        in0=xwork[:, c0:c0 + NFREE // 8],
                        in1=dfc[:], op=OP.mult)
                nc.vector.tensor_scalar(out=xwork[:], in0=xwork[:],
                                        scalar1=bs[:], scalar2=0.0,
                                        op0=OP.add, op1=OP.max)
                nc.vector.memset(xwork[:, NPC:], 0.0)

            pool = small.tile([32, 1], F32, name="pool")
            nc.vector.tensor_reduce(out=pool[:], in_=xwork[:],
                                    axis=mybir.AxisListType.X, op=OP.add)
            nc.sync.dma_start(out=pr_in[:, :], in_=pool[:])
            nc.gpsimd.collective_compute(
                "AllReduce", OP.add, replica_groups=[list(range(NCORES))],
                ins=[pr_in[:, :]], outs=[pr_out[:, :]])
            xcT = small.tile([96, 1], F32, name="xcT")
            nc.sync.dma_start(out=xcT[0:32, :], in_=pr_out[:, :])
            nc.vector.tensor_scalar(out=xcT[0:32, :], in0=xcT[0:32, :],
                                    scalar1=1.0 / N_NODES, scalar2=None,
                                    op0=OP.mult)
            nc.sync.dma_start(out=xcT[32:96, :], in_=d["xstate_c"][:, :])
            c0t = small.tile([128, 2], F32, name="c0t_sb")
            bhx = small.tile([128, 8], F32, name="bhx_sb")
            nc.sync.dma_start(out=c0t[:], in_=d["c0t"][:, :])
            nc.sync.dma_start(out=bhx[:], in_=d["bhx"][:, :])
            h0a = small.tile([128, 1], F32, name="h0a")
            h0b = small.tile([128, 1], F32, name="h0b")
            nc.sync.dma_start(out=h0a[:], in_=d["h0c"][0:128, :])
            nc.sync.dma_start(out=h0b[:], in_=d["h0c"][128:256, :])
            wihT = small.tile([96, 1024], F32, name="wihT_sb")
            whh0 = small.tile([128, 1024], F32, name="whh0_sb")
            whh1 = small.tile([128, 1024], F32, name="whh1_sb")
            nc.sync.dma_start(out=wihT[:], in_=d["wihT"][:, :])
            nc.sync.dma_start(out=whh0[:], in_=d["whh0"][:, :])
            nc.sync.dma_start(out=whh1[:], in_=d["whh1"][:, :])
            gps = psn.tile([128, 8], F32, name="gps", tag="gps")
            for m in range(8):
                nc.tensor.matmul(out=gps[:, m:m + 1],
                                 lhsT=wihT[:, m * 128:(m + 1) * 128],
                                 rhs=xcT[:], start=True, stop=False)
                nc.tensor.matmul(out=gps[:, m:m + 1],
                                 lhsT=whh0[:, m * 128:(m + 1) * 128],
                                 rhs=h0a[:], start=False, stop=False)
                nc.tensor.matmul(out=gps[:, m:m + 1],
                                 lhsT=whh1[:, m * 128:(m + 1) * 128],
                                 rhs=h0b[:], start=False, stop=True)
            gates = small.tile([128, 8], F32, name="gates")
            nc.vector.tensor_copy(out=gates[:], in_=gps[:])
            nc.vector.tensor_tensor(out=gates[:], in0=gates[:], in1=bhx[:],
                                    op=OP.add)
            act = small.tile([128, 8], F32, name="act")
            nc.scalar.activation(act[:, 0:4], gates[:, 0:4], AF.Sigmoid)
            nc.scalar.activation(act[:, 6:8], gates[:, 6:8], AF.Sigmoid)
            nc.scalar.activation(act[:, 4:6], gates[:, 4:6], AF.Tanh)
            cnew = small.tile([128, 2], F32, name="cnew")
            hnew = small.tile([128, 2], F32, name="hnew")
            tnk = small.tile([128, 2], F32, name="tnk")
            nc.vector.tensor_tensor(out=cnew[:], in0=act[:, 2:4], in1=c0t[:],
                                    op=OP.mult)
            nc.vector.tensor_tensor(out=tnk[:], in0=act[:, 0:2],
                                    in1=act[:, 4:6], op=OP.mult)
            nc.vector.tensor_tensor(out=cnew[:], in0=cnew[:], in1=tnk[:],
                                    op=OP.add)
            nc.scalar.activation(tnk[:], cnew[:], AF.Tanh)
            nc.vector.tensor_tensor(out=hnew[:], in0=act[:, 6:8], in1=tnk[:],
                                    op=OP.mult)
            wfc = small.tile([128, 2, 64], F32, name="wfc_sb")
            nc.sync.dma_start(out=wfc[:], in_=d["wfc"].ap().rearrange(
                "(t p) a -> p t a", p=128))
            pl = psn.tile([64, 1], F32, name="pl", tag="plog")
            for t2 in range(2):
                nc.tensor.matmul(out=pl[:], lhsT=wfc[:, t2, :],
                                 rhs=hnew[:, t2:t2 + 1],
                                 start=(t2 == 0), stop=(t2 == 1))
            lg64 = small.tile([64, 1], F32, name="lg64")
            nc.vector.tensor_copy(out=lg64[:], in_=pl[:])
            ident = small.tile([64, 64], F32, name="ident_sb")
            nc.sync.dma_start(out=ident[:], in_=d["ident"][:, :])
            pt = psn.tile([1, 64], F32, name="pt", tag="ptr")
            nc.tensor.matmul(out=pt[:], lhsT=lg64[:], rhs=ident[:],
                             start=True, stop=True)
            lrow = small.tile([1, 64], F32, name="lrow")
            bfc = small.tile([1, 64], F32, name="bfc_sb")
            nc.sync.dma_start(out=bfc[:], in_=d["bfc"][:, :])
            nc.vector.tensor_tensor(out=lrow[:], in0=pt[:], in1=bfc[:],
                                    op=OP.add)
            mx = small.tile([1, 1], F32, name="mx")
            nc.vector.tensor_reduce(out=mx[:], in_=lrow[:],
                                    axis=mybir.AxisListType.X, op=OP.max)
            nc.vector.tensor_scalar(out=lrow[:], in0=lrow[:], scalar1=mx[:],
                                    scalar2=None, op0=OP.subtract)
            ex = small.tile([1, 64], F32, name="ex")
            nc.scalar.activation(ex[:], lrow[:], AF.Exp)
            sm = small.tile([1, 1], F32, name="sm")
            nc.vector.tensor_reduce(out=sm[:], in_=ex[:],
                                    axis=mybir.AxisListType.X, op=OP.add)
            lsm = small.tile([1, 1], F32, name="lsm")
            nc.scalar.activation(lsm[:], sm[:], AF.Ln)
            nc.vector.tensor_scalar(out=lrow[:], in0=lrow[:], scalar1=lsm[:],
                                    scalar2=None, op0=OP.subtract)
            nc.sync.dma_start(out=out_lp[:, :], in_=lrow[:])
            nc.sync.dma_start(
                out=out_h.ap().rearrange("a (t p) -> a p t", p=128)[0],
                in_=hnew[:])
            nc.sync.dma_start(
                out=out_c.ap().rearrange("a (t p) -> a p t", p=128)[0],
                in_=cnew[:])
    nc.compile()
    return nc
